# revision 10
# baseline (speedup 1.0000x reference)
# Trainium2 Bass kernel for nn_CustomAttention (fused qkv + LoRA + per-head
# LayerNorm + softmax attention + output projection).
#
# Sharding: 16 heads split across 8 cores (2 heads/core), both batch elements
# on every core. Each core computes its heads' attention and its partial
# output projection; the host sums the 8 bf16 partials (f32 accumulate) and
# adds proj_b. LoRA is folded into the qkv weights on the host.
#
# v2 engine layout (per core):
#  - ACT runs ONLY exp (softmax) plus two tiny Ln/Exp calls per half for
#    rstd = exp(-0.5*ln(var+eps)); both functions live in the same activation
#    table set so the table is loaded once (Sqrt would thrash the table).
#  - DVE owns all PSUM evacuations (qkv stage, qT/kT, proj output), LN stats
#    reductions, and the softmax normalization (reciprocal + multiply).
#  - Pool (gpsimd) applies the LN normalization (tensor_scalar) and builds
#    the [v|ones] tiles; it never touches PSUM.
#  - PE: qkv matmuls, q/k transposes (col-tile paired), scores (row-tile
#    paired across the 2 heads: head0 rows 0-63, head1 rows 64-127 of the
#    array run concurrently), attention@[v|ones], packed 128-contraction
#    projection.
#  - Phase A (qkv+LN+transpose) of batch b+1 is emitted interleaved into
#    phase B (attention) of batch b so PE/DVE fill the gaps while ACT grinds
#    exp, and PE never idles long enough to lose the HAM clock boost.
import numpy as np
import ml_dtypes

import concourse.bass as bass
import concourse.bacc as bacc
import concourse.mybir as mybir
from concourse.tile import TileContext
from concourse.masks import make_identity
from concourse.bass_utils import run_bass_kernel_spmd

BF16 = ml_dtypes.bfloat16
F32 = np.float32

B, N, DIM, H, R = 2, 2048, 1024, 16, 8
D = DIM // H              # 64
NCORES = 8
HPC = H // NCORES         # 2 heads per core
ALPHA = 8.0
LORA_SCALE = ALPHA / R
EPS = 1e-5
QSCALE = float(D) ** -0.5  # 0.125

NCH = DIM // 128          # 8 contraction chunks of 128
NTI = N // 128            # 16 row tiles of 128
QI = 512                  # query-block width (free dim of score matmuls)
NIB = N // QI             # 4 query blocks per batch
HT = NTI // 2             # 8 row tiles per LN-stats half

_prog_cache: dict = {}


def _build_program(use_mask: bool, affine_q: bool, affine_k: bool, repeat: int = 1):
    nc = bacc.Bacc("TRN2", target_bir_lowering=False)
    f32 = mybir.dt.float32
    bf16 = mybir.dt.bfloat16

    xT = nc.dram_tensor("xT", [128, B, NCH, N], bf16, kind="ExternalInput")
    wT = nc.dram_tensor("wT", [NCH, 128, 6 * D], bf16, kind="ExternalInput")
    projT = nc.dram_tensor("projT", [128, DIM], bf16, kind="ExternalInput")
    out_p = nc.dram_tensor("out_p", [128, B, NTI, DIM], bf16, kind="ExternalOutput")
    if affine_q or affine_k:
        lnaff = nc.dram_tensor("lnaff", [4, 128, D], f32, kind="ExternalInput")
    if use_mask:
        emaskT = nc.dram_tensor("emaskT", [N, N], bf16, kind="ExternalInput")

    with TileContext(nc) as tc:
        import contextlib
        with contextlib.ExitStack() as ctx:
            const = ctx.enter_context(tc.tile_pool(name="const", bufs=1))
            ident = const.tile([128, 128], bf16)
            make_identity(nc, ident)
            eps_t = const.tile([128, 1], f32)
            nc.vector.memset(eps_t, EPS)
            # Preload the one activation-table set that holds BOTH Ln and
            # Exp; the table-load fixpoint then sees every Ln/Exp covered on
            # all paths and inserts no further (thrashing) loads.
            from concourse.hw_specs import get_activation_tables
            try:
                _tnames = list(get_activation_tables(nc.m.arch).keys())
                _set_id = _tnames.index("natural_log_exp_and_others")
            except Exception:
                _set_id = 6
            nc.scalar.add_instruction(
                mybir.InstLoadActFuncSet(
                    name=nc.get_next_instruction_name(), ins=[], outs=[],
                    act_func_set_id=_set_id))

            persist = ctx.enter_context(tc.tile_pool(name="persist", bufs=1))
            w_sb = persist.tile([128, NCH, 6 * D], bf16)
            nc.sync.dma_start(out=w_sb, in_=wT.rearrange("ci cm w -> cm ci w"))
            proj_sb = persist.tile([128, DIM], bf16)
            nc.sync.dma_start(out=proj_sb, in_=projT[:, :])
            if affine_q or affine_k:
                aff_sb = persist.tile([128, 4, D], f32)
                nc.sync.dma_start(out=aff_sb, in_=lnaff.rearrange("r p d -> p r d"))
            # [v|ones] stationary tiles: head0 = [v, 1], head1 = [1, v] so the
            # attention output of head1 lands on partitions 64-127 (den on
            # 0-63) and both heads' normalized outputs pack into one
            # 128-partition lhsT for the projection matmul.
            vps = []
            for par in range(B):
                vp = persist.tile([128, NTI, HPC, 128], bf16, name=f"vp{par}")
                nc.vector.memset(vp[:, :, 0, D:], 1.0)
                nc.vector.memset(vp[:, :, 1, :D], 1.0)
                vps.append(vp)

            xpool = ctx.enter_context(tc.tile_pool(name="xpool", bufs=2))
            qkpool = ctx.enter_context(tc.tile_pool(name="qkpool", bufs=2))
            stg = ctx.enter_context(tc.tile_pool(name="stg", bufs=2))
            sqp = ctx.enter_context(tc.tile_pool(name="sqp", bufs=2))
            lnp = ctx.enter_context(tc.tile_pool(name="lnp", bufs=2))
            natp = ctx.enter_context(tc.tile_pool(name="natp", bufs=8))
            esp = ctx.enter_context(tc.tile_pool(name="esp", bufs=4))
            otp = ctx.enter_context(tc.tile_pool(name="otp", bufs=4))
            outp = ctx.enter_context(tc.tile_pool(name="outp", bufs=2))
            if use_mask:
                mskp = ctx.enter_context(tc.tile_pool(name="mskp", bufs=4))
            # PSUM budget (8 banks, per-tag): sT f32 2x2 + av/pp shared 2 +
            # pq 1 + pt 1 = 8
            psS = ctx.enter_context(tc.tile_pool(name="psS", bufs=2, space="PSUM"))
            psB = ctx.enter_context(tc.tile_pool(name="psB", bufs=2, space="PSUM"))
            psA = ctx.enter_context(tc.tile_pool(name="psA", bufs=2, space="PSUM"))

            if repeat > 1:
                ctx.enter_context(tc.For_i(
                    0, repeat, 1,
                    hint_engines=(mybir.EngineType.PE, mybir.EngineType.SP,
                                  mybir.EngineType.Activation,
                                  mybir.EngineType.DVE, mybir.EngineType.Pool)))

            # ---- hoisted input loads for both batches ----
            x_sbs = []
            for b in range(B):
                x_sb = xpool.tile([128, NCH, N], bf16, tag="x_sb")
                nc.sync.dma_start(out=x_sb[:, 0:NCH // 2, :],
                                  in_=xT[:, b, 0:NCH // 2, :])
                nc.sync.dma_start(out=x_sb[:, NCH // 2:, :],
                                  in_=xT[:, b, NCH // 2:, :])
                x_sbs.append(x_sb)

            # ---------------- phase A emission (as fine-grained steps) -----
            def make_A_steps(b):
                """Returns (steps, handles); handles filled as steps run."""
                hd = {}
                x_sb = x_sbs[b]

                def alloc():
                    qkT = qkpool.tile([128, 2, N], bf16, tag="qkT", name="qkT")
                    hd["qT"] = qkT[:, 0, :]
                    hd["kT"] = qkT[:, 1, :]
                    hd["qkT"] = qkT
                    hd["stage"] = stg.tile([128, NTI, 6 * D], bf16, tag="stage", name="stage")
                    hd["sqs"] = sqp.tile([128, NTI, 6 * D], bf16, tag="sqs", name="sqs")

                def qkv_ti(ti):
                    def f():
                        stage = hd["stage"]; sqs = hd["sqs"]
                        pq = psA.tile([128, 512], f32, tag="pq", bufs=1)
                        for ci in range(NCH):
                            nc.tensor.matmul(
                                pq[:, 0:6 * D],
                                lhsT=x_sb[:, ci, ti * 128:(ti + 1) * 128],
                                rhs=w_sb[:, ci, :],
                                start=(ci == 0), stop=(ci == NCH - 1),
                            )
                        nc.vector.tensor_copy(out=stage[:, ti, :], in_=pq[:, 0:6 * D])
                        nc.vector.tensor_tensor(
                            out=sqs[:, ti, :], in0=stage[:, ti, :],
                            in1=stage[:, ti, :], op=mybir.AluOpType.mult)
                    return f

                def stats(half):
                    def f():
                        stage = hd["stage"]; sqs = hd["sqs"]
                        st6v = stage.rearrange("p t (i d) -> p t i d", d=D)
                        sq6v = sqs.rearrange("p t (i d) -> p t i d", d=D)
                        hsl = slice(half * HT, (half + 1) * HT)
                        mean = lnp.tile([128, HT, 6], bf16, tag="meanh")
                        with nc.allow_low_precision(reason="LN stats: bf16 out, fp32 internal accum"):
                            nc.vector.tensor_reduce(
                                out=mean, in_=st6v[:, hsl], axis=mybir.AxisListType.X,
                                op=mybir.AluOpType.add)
                        meanf = lnp.tile([128, HT, 6], f32, tag="mean")
                        nc.vector.tensor_scalar(
                            out=meanf, in0=mean, scalar1=1.0 / D, scalar2=None,
                            op0=mybir.AluOpType.mult)
                        varh = lnp.tile([128, HT, 6], bf16, tag="varh")
                        with nc.allow_low_precision(reason="LN stats: bf16 out, fp32 internal accum"):
                            nc.vector.tensor_reduce(
                                out=varh, in_=sq6v[:, hsl], axis=mybir.AxisListType.X,
                                op=mybir.AluOpType.add)
                        var = lnp.tile([128, HT, 6], f32, tag="var")
                        nc.vector.tensor_scalar(
                            out=var, in0=varh, scalar1=1.0 / D, scalar2=None,
                            op0=mybir.AluOpType.mult)
                        m2 = lnp.tile([128, HT, 6], f32, tag="m2")
                        nc.vector.tensor_tensor(
                            out=m2, in0=meanf, in1=meanf, op=mybir.AluOpType.mult)
                        nc.vector.tensor_tensor(
                            out=var, in0=var, in1=m2, op=mybir.AluOpType.subtract)
                        # rstd = exp(-0.5 * ln(var + eps)); Ln and Exp share
                        # one activation table set (unlike Sqrt).
                        lnv = lnp.tile([128, HT, 6], f32, tag="lnv")
                        nc.scalar.activation(
                            out=lnv, in_=var,
                            func=mybir.ActivationFunctionType.Ln,
                            bias=eps_t, scale=1.0)
                        rstd = lnp.tile([128, HT, 6], f32, tag="rstd")
                        nc.scalar.activation(
                            out=rstd, in_=lnv,
                            func=mybir.ActivationFunctionType.Exp,
                            scale=-0.5)
                        if not affine_q:
                            nc.vector.tensor_scalar(
                                out=rstd[:, :, 0:6:3], in0=rstd[:, :, 0:6:3],
                                scalar1=QSCALE, scalar2=None,
                                op0=mybir.AluOpType.mult)
                        hd[("mean", half)] = meanf
                        hd[("rstd", half)] = rstd
                    return f

                def natT_ti(half, tih):
                    def f():
                        stage = hd["stage"]
                        st6v = stage.rearrange("p t (i d) -> p t i d", d=D)
                        mean = hd[("mean", half)]; rstd = hd[("rstd", half)]
                        ti = half * HT + tih
                        nats = []
                        # order q_h0, q_h1, k_h0, k_h1 so the two transposes
                        # of each pair hit col-groups (0,*) and (64,*)
                        for inst, qk, hh in ((0, 0, 0), (3, 0, 1),
                                             (1, 1, 0), (4, 1, 1)):
                            affine = affine_q if qk == 0 else affine_k
                            nat = natp.tile([128, D], bf16, tag="nat")
                            if affine:
                                natf = natp.tile([128, D], f32, tag="natf")
                                nc.gpsimd.tensor_scalar(
                                    out=natf, in0=st6v[:, ti, inst, :],
                                    scalar1=mean[:, tih, inst:inst + 1],
                                    scalar2=rstd[:, tih, inst:inst + 1],
                                    op0=mybir.AluOpType.subtract,
                                    op1=mybir.AluOpType.mult)
                                r = 0 if qk == 0 else 2
                                natf2 = natp.tile([128, D], f32, tag="natf2")
                                nc.gpsimd.tensor_tensor(
                                    out=natf2, in0=natf, in1=aff_sb[:, r, :],
                                    op=mybir.AluOpType.mult)
                                nc.gpsimd.tensor_tensor(
                                    out=nat, in0=natf2, in1=aff_sb[:, r + 1, :],
                                    op=mybir.AluOpType.add)
                            else:
                                nc.gpsimd.tensor_scalar(
                                    out=nat, in0=st6v[:, ti, inst, :],
                                    scalar1=mean[:, tih, inst:inst + 1],
                                    scalar2=rstd[:, tih, inst:inst + 1],
                                    op0=mybir.AluOpType.subtract,
                                    op1=mybir.AluOpType.mult)
                            nats.append((nat, qk, hh))
                        pt = psA.tile([128, 2, 512], bf16, tag="pt", bufs=1)
                        for nat, qk, hh in nats:
                            nc.tensor.transpose(
                                pt[hh * D:(hh + 1) * D, qk, 0:128], nat, ident)
                        nc.vector.tensor_copy(
                            out=hd["qkT"][:, :, ti * 128:(ti + 1) * 128],
                            in_=pt[:, :, 0:128])
                        nc.gpsimd.tensor_copy(
                            out=vps[b][:, ti, 0, 0:D], in_=st6v[:, ti, 2, :])
                        nc.gpsimd.tensor_copy(
                            out=vps[b][:, ti, 1, D:], in_=st6v[:, ti, 5, :])
                    return f

                steps = [alloc]
                for half in range(2):
                    for tih in range(HT):
                        steps.append(qkv_ti(half * HT + tih))
                    steps.append(stats(half))
                # natT after BOTH halves' qkv so the PE stream never gaps
                # on the (DVE-only) stats steps long enough to lose HAM
                for half in range(2):
                    for tih in range(HT):
                        steps.append(natT_ti(half, tih))
                return steps, hd

            def run_all(steps):
                for s in steps:
                    s()

            # ---------------- phase B emission ------------------------------
            def emit_B(b, hd, filler):
                def pull(n=1):
                    for _ in range(n):
                        s = next(filler, None)
                        if s is not None:
                            s()
                qT = hd["qT"]; kT = hd["kT"]; vp = vps[b]
                def emit_scores(ib, jp):
                    i0 = ib * QI
                    sTs = []
                    for hh in range(HPC):
                        sTs.append(psS.tile([128, 2, QI], f32, tag="sT", name="sT"))
                    for cj in range(2):
                        j = jp * 2 + cj
                        for hh in range(HPC):
                            hs = slice(hh * D, (hh + 1) * D)
                            nc.tensor.matmul(
                                sTs[hh][:, cj, :],
                                lhsT=kT[hs, j * 128:(j + 1) * 128],
                                rhs=qT[hs, i0:i0 + QI],
                                start=True, stop=True,
                            )
                    return sTs

                for ib in range(NIB):
                    i0 = ib * QI
                    avs = []
                    for hh in range(HPC):
                        av = psB.tile([128, QI], f32, tag="av", name="av")
                        avs.append(av)
                    sTs = emit_scores(ib, 0)
                    for jp in range(8):
                        ess = []
                        for hh in range(HPC):
                            ess.append(esp.tile([128, 2, QI], bf16, tag="es", name="es"))
                        for hh in range(HPC):
                            nc.scalar.activation(
                                out=ess[hh], in_=sTs[hh],
                                func=mybir.ActivationFunctionType.Exp)
                        if use_mask:
                            for hh in range(HPC):
                                msk = mskp.tile([128, 2, QI], bf16, tag="msk")
                                for cj in range(2):
                                    j = jp * 2 + cj
                                    nc.sync.dma_start(
                                        out=msk[:, cj, :],
                                        in_=emaskT[j * 128:(j + 1) * 128,
                                                   i0:i0 + QI])
                                nc.vector.tensor_tensor(
                                    out=ess[hh], in0=ess[hh], in1=msk,
                                    op=mybir.AluOpType.mult)
                        # next jp's scores go on the PE queue AHEAD of the
                        # exp-gated av matmuls so ACT never starves
                        if jp + 1 < 8:
                            sTs = emit_scores(ib, jp + 1)
                        pull(1)
                        for hh in range(HPC):
                            for cj in range(2):
                                j = jp * 2 + cj
                                nc.tensor.matmul(
                                    avs[hh],
                                    lhsT=vp[:, j, hh, :],
                                    rhs=ess[hh][:, cj, :],
                                    start=(j == 0), stop=(j == NTI - 1),
                                )
                    # normalization: av0 = [out|den], av1 = [den|out]
                    zr = otp.tile([128, QI], f32, tag="zr")
                    nc.vector.reciprocal(out=zr[0:D, :], in_=avs[0][D:, :])
                    nc.vector.reciprocal(out=zr[D:, :], in_=avs[1][0:D, :])
                    oT2 = otp.tile([128, QI], bf16, tag="oT2")
                    nc.vector.tensor_tensor(
                        out=oT2[0:D, :], in0=avs[0][0:D, :], in1=zr[0:D, :],
                        op=mybir.AluOpType.mult)
                    nc.vector.tensor_tensor(
                        out=oT2[D:, :], in0=avs[1][D:, :], in1=zr[D:, :],
                        op=mybir.AluOpType.mult)
                    pull(1)
                    osb = outp.tile([128, QI // 128, DIM], bf16, tag="osb")
                    for sub in range(QI // 128):
                        for nh in range(2):
                            pp = psB.tile([128, 512], f32, tag="av", name="pp")
                            nc.tensor.matmul(
                                pp,
                                lhsT=oT2[:, sub * 128:(sub + 1) * 128],
                                rhs=proj_sb[:, nh * 512:(nh + 1) * 512],
                                start=True, stop=True,
                            )
                            nc.vector.tensor_copy(
                                out=osb[:, sub, nh * 512:(nh + 1) * 512], in_=pp)
                        pull(1)
                    ti0 = ib * (QI // 128)
                    nc.sync.dma_start(
                        out=out_p[:, b, ti0:ti0 + QI // 128, :], in_=osb)

            # ---------------- schedule --------------------------------------
            stepsA0, hd0 = make_A_steps(0)
            run_all(stepsA0)
            stepsA1, hd1 = make_A_steps(1)
            emit_B(0, hd0, iter(stepsA1))
            emit_B(1, hd1, iter(()))
    nc.compile()
    return nc


def _prep_inputs(inputs):
    x = np.ascontiguousarray(inputs["x"], dtype=F32)
    qkv_w = np.asarray(inputs["qkv_w"], dtype=F32)
    proj_w = np.asarray(inputs["proj_w"], dtype=F32)
    W_eff = qkv_w.copy()
    for i, (a, bm) in enumerate([("lora_Aq", "lora_Bq"), ("lora_Ak", "lora_Bk"),
                                 ("lora_Av", "lora_Bv")]):
        A = np.asarray(inputs[a], dtype=F32)
        Bm = np.asarray(inputs[bm], dtype=F32)
        W_eff[i * DIM:(i + 1) * DIM] += LORA_SCALE * (A @ Bm).T

    xT_all = np.ascontiguousarray(
        x.transpose(2, 0, 1).reshape(NCH, 128, B, N)
        .transpose(1, 2, 0, 3).astype(BF16))

    qn_w = np.asarray(inputs["qn_w"], F32); qn_b = np.asarray(inputs["qn_b"], F32)
    kn_w = np.asarray(inputs["kn_w"], F32); kn_b = np.asarray(inputs["kn_b"], F32)
    affine_q = not (np.all(qn_w == 1.0) and np.all(qn_b == 0.0))
    affine_k = not (np.all(kn_w == 1.0) and np.all(kn_b == 0.0))
    mask = np.asarray(inputs["attn_mask"], F32)
    use_mask = bool(np.any(mask))

    common = {"xT": xT_all}
    if affine_q or affine_k:
        aff = np.stack([
            np.broadcast_to(qn_w * QSCALE, (128, D)),
            np.broadcast_to(qn_b * QSCALE, (128, D)),
            np.broadcast_to(kn_w, (128, D)),
            np.broadcast_to(kn_b, (128, D)),
        ]).astype(F32)
        common["lnaff"] = np.ascontiguousarray(aff)
    if use_mask:
        common["emaskT"] = np.ascontiguousarray(
            np.exp(mask[0, 0].T).astype(BF16))

    in_maps = []
    for c in range(NCORES):
        h0 = c * HPC
        blocks = []
        for hh in range(HPC):
            h = h0 + hh
            for part in range(3):  # q, k, v
                blocks.append(W_eff[part * DIM + h * D: part * DIM + (h + 1) * D])
        Wlocal = np.concatenate(blocks, axis=0)          # [384, 1024]
        wT_c = np.ascontiguousarray(
            Wlocal.T.reshape(NCH, 128, 6 * D).astype(BF16))
        projT_c = np.ascontiguousarray(np.concatenate(
            [proj_w[:, (h0 + hh) * D:(h0 + hh + 1) * D].T for hh in range(HPC)],
            axis=0).astype(BF16))                        # [128, 1024]
        m = dict(common)
        m["wT"] = wT_c
        m["projT"] = projT_c
        in_maps.append(m)
    return in_maps, (use_mask, affine_q, affine_k)


def _run(inputs, trace=False):
    in_maps, key = _prep_inputs(inputs)
    if key not in _prog_cache:
        _prog_cache[key] = _build_program(*key)
    nc = _prog_cache[key]
    res = run_bass_kernel_spmd(nc, in_maps, core_ids=list(range(NCORES)),
                               trace=trace)
    acc = np.zeros((128, B, NTI, DIM), dtype=F32)
    for r in res.results:
        acc += r["out_p"].astype(F32)
    out = np.ascontiguousarray(acc.transpose(1, 2, 0, 3).reshape(B, N, DIM))
    out += np.asarray(inputs["proj_b"], F32)
    return out, res


def kernel(**inputs) -> np.ndarray:
    out, _ = _run(inputs)
    return out


# revision 11
# speedup vs baseline: 1.0488x; 1.0488x over previous
# Trainium2 Bass kernel for nn_CustomAttention (fused qkv + LoRA + per-head
# LayerNorm + softmax attention + output projection).
#
# Sharding: 16 heads split across 8 cores (2 heads/core), both batch elements
# on every core. Each core computes its heads' attention and its partial
# output projection; the host sums the 8 bf16 partials (f32 accumulate) and
# adds proj_b. LoRA is folded into the qkv weights on the host.
#
# v2 engine layout (per core):
#  - ACT runs ONLY exp (softmax) plus two tiny Ln/Exp calls per half for
#    rstd = exp(-0.5*ln(var+eps)); both functions live in the same activation
#    table set so the table is loaded once (Sqrt would thrash the table).
#  - DVE owns all PSUM evacuations (qkv stage, qT/kT, proj output), LN stats
#    reductions, and the softmax normalization (reciprocal + multiply).
#  - Pool (gpsimd) applies the LN normalization (tensor_scalar) and builds
#    the [v|ones] tiles; it never touches PSUM.
#  - PE: qkv matmuls, q/k transposes (col-tile paired), scores (row-tile
#    paired across the 2 heads: head0 rows 0-63, head1 rows 64-127 of the
#    array run concurrently), attention@[v|ones], packed 128-contraction
#    projection.
#  - Phase A (qkv+LN+transpose) of batch b+1 is emitted interleaved into
#    phase B (attention) of batch b so PE/DVE fill the gaps while ACT grinds
#    exp, and PE never idles long enough to lose the HAM clock boost.
import numpy as np
import ml_dtypes

import concourse.bass as bass
import concourse.bacc as bacc
import concourse.mybir as mybir
from concourse.tile import TileContext
from concourse.masks import make_identity
from concourse.bass_utils import run_bass_kernel_spmd

BF16 = ml_dtypes.bfloat16
F32 = np.float32

B, N, DIM, H, R = 2, 2048, 1024, 16, 8
D = DIM // H              # 64
NCORES = 8
HPC = H // NCORES         # 2 heads per core
ALPHA = 8.0
LORA_SCALE = ALPHA / R
EPS = 1e-5
QSCALE = float(D) ** -0.5  # 0.125

NCH = DIM // 128          # 8 contraction chunks of 128
NTI = N // 128            # 16 row tiles of 128
QI = 512                  # query-block width (free dim of score matmuls)
NIB = N // QI             # 4 query blocks per batch
HT = NTI // 2             # 8 row tiles per LN-stats half

_prog_cache: dict = {}


def _build_program(use_mask: bool, affine_q: bool, affine_k: bool, repeat: int = 1):
    nc = bacc.Bacc("TRN2", target_bir_lowering=False)
    f32 = mybir.dt.float32
    bf16 = mybir.dt.bfloat16

    xT = nc.dram_tensor("xT", [128, B, NCH, N], bf16, kind="ExternalInput")
    wT = nc.dram_tensor("wT", [NCH, 128, 6 * D], bf16, kind="ExternalInput")
    projT = nc.dram_tensor("projT", [128, DIM], bf16, kind="ExternalInput")
    out_p = nc.dram_tensor("out_p", [128, B, NTI, DIM], bf16, kind="ExternalOutput")
    if affine_q or affine_k:
        lnaff = nc.dram_tensor("lnaff", [4, 128, D], f32, kind="ExternalInput")
    if use_mask:
        emaskT = nc.dram_tensor("emaskT", [N, N], bf16, kind="ExternalInput")

    with TileContext(nc) as tc:
        import contextlib
        with contextlib.ExitStack() as ctx:
            const = ctx.enter_context(tc.tile_pool(name="const", bufs=1))
            ident = const.tile([128, 128], bf16)
            make_identity(nc, ident)
            eps_t = const.tile([128, 1], f32)
            nc.vector.memset(eps_t, EPS)
            # Preload the one activation-table set that holds BOTH Ln and
            # Exp; the table-load fixpoint then sees every Ln/Exp covered on
            # all paths and inserts no further (thrashing) loads.
            from concourse.hw_specs import get_activation_tables
            try:
                _tnames = list(get_activation_tables(nc.m.arch).keys())
                _set_id = _tnames.index("natural_log_exp_and_others")
            except Exception:
                _set_id = 6
            nc.scalar.add_instruction(
                mybir.InstLoadActFuncSet(
                    name=nc.get_next_instruction_name(), ins=[], outs=[],
                    act_func_set_id=_set_id))

            persist = ctx.enter_context(tc.tile_pool(name="persist", bufs=1))
            w_sb = persist.tile([128, NCH, 6 * D], bf16)
            nc.sync.dma_start(out=w_sb, in_=wT.rearrange("ci cm w -> cm ci w"))
            proj_sb = persist.tile([128, DIM], bf16)
            nc.sync.dma_start(out=proj_sb, in_=projT[:, :])
            if affine_q or affine_k:
                aff_sb = persist.tile([128, 4, D], f32)
                nc.sync.dma_start(out=aff_sb, in_=lnaff.rearrange("r p d -> p r d"))
            # [v|ones] stationary tiles: head0 = [v, 1], head1 = [1, v] so the
            # attention output of head1 lands on partitions 64-127 (den on
            # 0-63) and both heads' normalized outputs pack into one
            # 128-partition lhsT for the projection matmul.
            vps = []
            for par in range(B):
                vp = persist.tile([128, NTI, HPC, 128], bf16, name=f"vp{par}")
                nc.vector.memset(vp[:, :, 0, D:], 1.0)
                nc.vector.memset(vp[:, :, 1, :D], 1.0)
                vps.append(vp)

            xpool = ctx.enter_context(tc.tile_pool(name="xpool", bufs=2))
            qkpool = ctx.enter_context(tc.tile_pool(name="qkpool", bufs=2))
            stg = ctx.enter_context(tc.tile_pool(name="stg", bufs=2))
            sqp = ctx.enter_context(tc.tile_pool(name="sqp", bufs=2))
            lnp = ctx.enter_context(tc.tile_pool(name="lnp", bufs=2))
            natp = ctx.enter_context(tc.tile_pool(name="natp", bufs=8))
            esp = ctx.enter_context(tc.tile_pool(name="esp", bufs=4))
            otp = ctx.enter_context(tc.tile_pool(name="otp", bufs=4))
            outp = ctx.enter_context(tc.tile_pool(name="outp", bufs=2))
            if use_mask:
                mskp = ctx.enter_context(tc.tile_pool(name="mskp", bufs=4))
            # PSUM budget (8 banks, per-tag): sT f32 2x2 + av/pp shared 2 +
            # pq 1 + pt 1 = 8
            psS = ctx.enter_context(tc.tile_pool(name="psS", bufs=2, space="PSUM"))
            psB = ctx.enter_context(tc.tile_pool(name="psB", bufs=2, space="PSUM"))
            psA = ctx.enter_context(tc.tile_pool(name="psA", bufs=2, space="PSUM"))

            if repeat > 1:
                ctx.enter_context(tc.For_i(
                    0, repeat, 1,
                    hint_engines=(mybir.EngineType.PE, mybir.EngineType.SP,
                                  mybir.EngineType.Activation,
                                  mybir.EngineType.DVE, mybir.EngineType.Pool)))

            # ---- hoisted input loads for both batches ----
            x_sbs = []
            for b in range(B):
                x_sb = xpool.tile([128, NCH, N], bf16, tag="x_sb")
                nc.sync.dma_start(out=x_sb[:, 0:NCH // 2, :],
                                  in_=xT[:, b, 0:NCH // 2, :])
                nc.sync.dma_start(out=x_sb[:, NCH // 2:, :],
                                  in_=xT[:, b, NCH // 2:, :])
                x_sbs.append(x_sb)

            # ---------------- phase A emission (as fine-grained steps) -----
            def make_A_steps(b, borrow_sT=False):
                """Returns (steps, handles); handles filled as steps run.
                borrow_sT: batch-0's phase A runs with no phase B interleaved
                under it, so its qkv accumulators borrow the (idle) sT psum
                tag to double-buffer instead of serializing on one bank."""
                hd = {}
                x_sb = x_sbs[b]

                def alloc():
                    qkT = qkpool.tile([128, 2, N], bf16, tag="qkT", name="qkT")
                    hd["qT"] = qkT[:, 0, :]
                    hd["kT"] = qkT[:, 1, :]
                    hd["qkT"] = qkT
                    hd["stage"] = stg.tile([128, NTI, 6 * D], bf16, tag="stage", name="stage")
                    hd["sqs"] = sqp.tile([128, NTI, 6 * D], bf16, tag="sqs", name="sqs")

                def qkv_ti(ti):
                    def f():
                        stage = hd["stage"]; sqs = hd["sqs"]
                        if borrow_sT:
                            pqt = psS.tile([128, 2, QI], f32, tag="sT", name="pqt")
                            pq = pqt[:, 0, :]
                        else:
                            pq = psA.tile([128, 512], f32, tag="pq", bufs=1)
                        for ci in range(NCH):
                            nc.tensor.matmul(
                                pq[:, 0:6 * D],
                                lhsT=x_sb[:, ci, ti * 128:(ti + 1) * 128],
                                rhs=w_sb[:, ci, :],
                                start=(ci == 0), stop=(ci == NCH - 1),
                            )
                        nc.vector.tensor_copy(out=stage[:, ti, :], in_=pq[:, 0:6 * D])
                        nc.vector.tensor_tensor(
                            out=sqs[:, ti, :], in0=stage[:, ti, :],
                            in1=stage[:, ti, :], op=mybir.AluOpType.mult)
                    return f

                def stats(half):
                    def f():
                        stage = hd["stage"]; sqs = hd["sqs"]
                        st6v = stage.rearrange("p t (i d) -> p t i d", d=D)
                        sq6v = sqs.rearrange("p t (i d) -> p t i d", d=D)
                        hsl = slice(half * HT, (half + 1) * HT)
                        mean = lnp.tile([128, HT, 6], bf16, tag="meanh")
                        with nc.allow_low_precision(reason="LN stats: bf16 out, fp32 internal accum"):
                            nc.vector.tensor_reduce(
                                out=mean, in_=st6v[:, hsl], axis=mybir.AxisListType.X,
                                op=mybir.AluOpType.add)
                        meanf = lnp.tile([128, HT, 6], f32, tag="mean")
                        nc.vector.tensor_scalar(
                            out=meanf, in0=mean, scalar1=1.0 / D, scalar2=None,
                            op0=mybir.AluOpType.mult)
                        varh = lnp.tile([128, HT, 6], bf16, tag="varh")
                        with nc.allow_low_precision(reason="LN stats: bf16 out, fp32 internal accum"):
                            nc.vector.tensor_reduce(
                                out=varh, in_=sq6v[:, hsl], axis=mybir.AxisListType.X,
                                op=mybir.AluOpType.add)
                        var = lnp.tile([128, HT, 6], f32, tag="var")
                        nc.vector.tensor_scalar(
                            out=var, in0=varh, scalar1=1.0 / D, scalar2=None,
                            op0=mybir.AluOpType.mult)
                        m2 = lnp.tile([128, HT, 6], f32, tag="m2")
                        nc.vector.tensor_tensor(
                            out=m2, in0=meanf, in1=meanf, op=mybir.AluOpType.mult)
                        nc.vector.tensor_tensor(
                            out=var, in0=var, in1=m2, op=mybir.AluOpType.subtract)
                        # rstd = exp(-0.5 * ln(var + eps)); Ln and Exp share
                        # one activation table set (unlike Sqrt).
                        lnv = lnp.tile([128, HT, 6], f32, tag="lnv")
                        nc.scalar.activation(
                            out=lnv, in_=var,
                            func=mybir.ActivationFunctionType.Ln,
                            bias=eps_t, scale=1.0)
                        rstd = lnp.tile([128, HT, 6], f32, tag="rstd")
                        nc.scalar.activation(
                            out=rstd, in_=lnv,
                            func=mybir.ActivationFunctionType.Exp,
                            scale=-0.5)
                        if not affine_q:
                            nc.vector.tensor_scalar(
                                out=rstd[:, :, 0:6:3], in0=rstd[:, :, 0:6:3],
                                scalar1=QSCALE, scalar2=None,
                                op0=mybir.AluOpType.mult)
                        hd[("mean", half)] = meanf
                        hd[("rstd", half)] = rstd
                    return f

                def natT_ti(half, tih):
                    def f():
                        stage = hd["stage"]
                        st6v = stage.rearrange("p t (i d) -> p t i d", d=D)
                        mean = hd[("mean", half)]; rstd = hd[("rstd", half)]
                        ti = half * HT + tih
                        nats = []
                        # order q_h0, q_h1, k_h0, k_h1 so the two transposes
                        # of each pair hit col-groups (0,*) and (64,*)
                        for inst, qk, hh in ((0, 0, 0), (3, 0, 1),
                                             (1, 1, 0), (4, 1, 1)):
                            affine = affine_q if qk == 0 else affine_k
                            nat = natp.tile([128, D], bf16, tag="nat")
                            if affine:
                                natf = natp.tile([128, D], f32, tag="natf")
                                nc.gpsimd.tensor_scalar(
                                    out=natf, in0=st6v[:, ti, inst, :],
                                    scalar1=mean[:, tih, inst:inst + 1],
                                    scalar2=rstd[:, tih, inst:inst + 1],
                                    op0=mybir.AluOpType.subtract,
                                    op1=mybir.AluOpType.mult)
                                r = 0 if qk == 0 else 2
                                natf2 = natp.tile([128, D], f32, tag="natf2")
                                nc.gpsimd.tensor_tensor(
                                    out=natf2, in0=natf, in1=aff_sb[:, r, :],
                                    op=mybir.AluOpType.mult)
                                nc.gpsimd.tensor_tensor(
                                    out=nat, in0=natf2, in1=aff_sb[:, r + 1, :],
                                    op=mybir.AluOpType.add)
                            else:
                                nc.gpsimd.tensor_scalar(
                                    out=nat, in0=st6v[:, ti, inst, :],
                                    scalar1=mean[:, tih, inst:inst + 1],
                                    scalar2=rstd[:, tih, inst:inst + 1],
                                    op0=mybir.AluOpType.subtract,
                                    op1=mybir.AluOpType.mult)
                            nats.append((nat, qk, hh))
                        pt = psA.tile([128, 2, 512], bf16, tag="pt", bufs=1)
                        for nat, qk, hh in nats:
                            nc.tensor.transpose(
                                pt[hh * D:(hh + 1) * D, qk, 0:128], nat, ident)
                        nc.vector.tensor_copy(
                            out=hd["qkT"][:, :, ti * 128:(ti + 1) * 128],
                            in_=pt[:, :, 0:128])
                        nc.gpsimd.tensor_copy(
                            out=vps[b][:, ti, 0, 0:D], in_=st6v[:, ti, 2, :])
                        nc.gpsimd.tensor_copy(
                            out=vps[b][:, ti, 1, D:], in_=st6v[:, ti, 5, :])
                    return f

                steps = [alloc]
                for half in range(2):
                    for tih in range(HT):
                        steps.append(qkv_ti(half * HT + tih))
                    steps.append(stats(half))
                # natT after BOTH halves' qkv so the PE stream never gaps
                # on the (DVE-only) stats steps long enough to lose HAM
                for half in range(2):
                    for tih in range(HT):
                        steps.append(natT_ti(half, tih))
                return steps, hd

            def run_all(steps):
                for s in steps:
                    s()

            # ---------------- phase B emission ------------------------------
            def emit_B(b, hd, filler):
                def pull(n=1):
                    for _ in range(n):
                        s = next(filler, None)
                        if s is not None:
                            s()
                qT = hd["qT"]; kT = hd["kT"]; vp = vps[b]
                def emit_scores(ib, jp):
                    i0 = ib * QI
                    sTs = []
                    for hh in range(HPC):
                        sTs.append(psS.tile([128, 2, QI], f32, tag="sT", name="sT"))
                    for cj in range(2):
                        j = jp * 2 + cj
                        for hh in range(HPC):
                            hs = slice(hh * D, (hh + 1) * D)
                            nc.tensor.matmul(
                                sTs[hh][:, cj, :],
                                lhsT=kT[hs, j * 128:(j + 1) * 128],
                                rhs=qT[hs, i0:i0 + QI],
                                start=True, stop=True,
                            )
                    return sTs

                for ib in range(NIB):
                    i0 = ib * QI
                    avs = []
                    for hh in range(HPC):
                        av = psB.tile([128, QI], f32, tag="av", name="av")
                        avs.append(av)
                    sTs = emit_scores(ib, 0)
                    for jp in range(8):
                        ess = []
                        for hh in range(HPC):
                            ess.append(esp.tile([128, 2, QI], bf16, tag="es", name="es"))
                        for hh in range(HPC):
                            nc.scalar.activation(
                                out=ess[hh], in_=sTs[hh],
                                func=mybir.ActivationFunctionType.Exp)
                        if use_mask:
                            for hh in range(HPC):
                                msk = mskp.tile([128, 2, QI], bf16, tag="msk")
                                for cj in range(2):
                                    j = jp * 2 + cj
                                    nc.sync.dma_start(
                                        out=msk[:, cj, :],
                                        in_=emaskT[j * 128:(j + 1) * 128,
                                                   i0:i0 + QI])
                                nc.vector.tensor_tensor(
                                    out=ess[hh], in0=ess[hh], in1=msk,
                                    op=mybir.AluOpType.mult)
                        # next jp's scores go on the PE queue AHEAD of the
                        # exp-gated av matmuls so ACT never starves
                        if jp + 1 < 8:
                            sTs = emit_scores(ib, jp + 1)
                        pull(1)
                        for hh in range(HPC):
                            for cj in range(2):
                                j = jp * 2 + cj
                                nc.tensor.matmul(
                                    avs[hh],
                                    lhsT=vp[:, j, hh, :],
                                    rhs=ess[hh][:, cj, :],
                                    start=(j == 0), stop=(j == NTI - 1),
                                )
                    # normalization: av0 = [out|den], av1 = [den|out]
                    zr = otp.tile([128, QI], f32, tag="zr")
                    nc.vector.reciprocal(out=zr[0:D, :], in_=avs[0][D:, :])
                    nc.vector.reciprocal(out=zr[D:, :], in_=avs[1][0:D, :])
                    oT2 = otp.tile([128, QI], bf16, tag="oT2")
                    nc.vector.tensor_tensor(
                        out=oT2[0:D, :], in0=avs[0][0:D, :], in1=zr[0:D, :],
                        op=mybir.AluOpType.mult)
                    nc.vector.tensor_tensor(
                        out=oT2[D:, :], in0=avs[1][D:, :], in1=zr[D:, :],
                        op=mybir.AluOpType.mult)
                    pull(1)
                    osb = outp.tile([128, QI // 128, DIM], bf16, tag="osb")
                    for sub in range(QI // 128):
                        for nh in range(2):
                            pp = psB.tile([128, 512], f32, tag="av", name="pp")
                            nc.tensor.matmul(
                                pp,
                                lhsT=oT2[:, sub * 128:(sub + 1) * 128],
                                rhs=proj_sb[:, nh * 512:(nh + 1) * 512],
                                start=True, stop=True,
                            )
                            nc.vector.tensor_copy(
                                out=osb[:, sub, nh * 512:(nh + 1) * 512], in_=pp)
                        pull(1)
                    ti0 = ib * (QI // 128)
                    nc.sync.dma_start(
                        out=out_p[:, b, ti0:ti0 + QI // 128, :], in_=osb)

            # ---------------- schedule --------------------------------------
            stepsA0, hd0 = make_A_steps(0, borrow_sT=True)
            run_all(stepsA0)
            stepsA1, hd1 = make_A_steps(1)
            emit_B(0, hd0, iter(stepsA1))
            emit_B(1, hd1, iter(()))
    nc.compile()
    return nc


def _prep_inputs(inputs):
    x = np.ascontiguousarray(inputs["x"], dtype=F32)
    qkv_w = np.asarray(inputs["qkv_w"], dtype=F32)
    proj_w = np.asarray(inputs["proj_w"], dtype=F32)
    W_eff = qkv_w.copy()
    for i, (a, bm) in enumerate([("lora_Aq", "lora_Bq"), ("lora_Ak", "lora_Bk"),
                                 ("lora_Av", "lora_Bv")]):
        A = np.asarray(inputs[a], dtype=F32)
        Bm = np.asarray(inputs[bm], dtype=F32)
        W_eff[i * DIM:(i + 1) * DIM] += LORA_SCALE * (A @ Bm).T

    xT_all = np.ascontiguousarray(
        x.transpose(2, 0, 1).reshape(NCH, 128, B, N)
        .transpose(1, 2, 0, 3).astype(BF16))

    qn_w = np.asarray(inputs["qn_w"], F32); qn_b = np.asarray(inputs["qn_b"], F32)
    kn_w = np.asarray(inputs["kn_w"], F32); kn_b = np.asarray(inputs["kn_b"], F32)
    affine_q = not (np.all(qn_w == 1.0) and np.all(qn_b == 0.0))
    affine_k = not (np.all(kn_w == 1.0) and np.all(kn_b == 0.0))
    mask = np.asarray(inputs["attn_mask"], F32)
    use_mask = bool(np.any(mask))

    common = {"xT": xT_all}
    if affine_q or affine_k:
        aff = np.stack([
            np.broadcast_to(qn_w * QSCALE, (128, D)),
            np.broadcast_to(qn_b * QSCALE, (128, D)),
            np.broadcast_to(kn_w, (128, D)),
            np.broadcast_to(kn_b, (128, D)),
        ]).astype(F32)
        common["lnaff"] = np.ascontiguousarray(aff)
    if use_mask:
        common["emaskT"] = np.ascontiguousarray(
            np.exp(mask[0, 0].T).astype(BF16))

    in_maps = []
    for c in range(NCORES):
        h0 = c * HPC
        blocks = []
        for hh in range(HPC):
            h = h0 + hh
            for part in range(3):  # q, k, v
                blocks.append(W_eff[part * DIM + h * D: part * DIM + (h + 1) * D])
        Wlocal = np.concatenate(blocks, axis=0)          # [384, 1024]
        wT_c = np.ascontiguousarray(
            Wlocal.T.reshape(NCH, 128, 6 * D).astype(BF16))
        projT_c = np.ascontiguousarray(np.concatenate(
            [proj_w[:, (h0 + hh) * D:(h0 + hh + 1) * D].T for hh in range(HPC)],
            axis=0).astype(BF16))                        # [128, 1024]
        m = dict(common)
        m["wT"] = wT_c
        m["projT"] = projT_c
        in_maps.append(m)
    return in_maps, (use_mask, affine_q, affine_k)


def _run(inputs, trace=False):
    in_maps, key = _prep_inputs(inputs)
    if key not in _prog_cache:
        _prog_cache[key] = _build_program(*key)
    nc = _prog_cache[key]
    res = run_bass_kernel_spmd(nc, in_maps, core_ids=list(range(NCORES)),
                               trace=trace)
    acc = np.zeros((128, B, NTI, DIM), dtype=F32)
    for r in res.results:
        acc += r["out_p"].astype(F32)
    out = np.ascontiguousarray(acc.transpose(1, 2, 0, 3).reshape(B, N, DIM))
    out += np.asarray(inputs["proj_b"], F32)
    return out, res


def kernel(**inputs) -> np.ndarray:
    out, _ = _run(inputs)
    return out


# revision 14
# speedup vs baseline: 1.1383x; 1.0854x over previous
# Trainium2 Bass kernel for nn_CustomAttention (fused qkv + LoRA + per-head
# LayerNorm + softmax attention + output projection).
#
# Sharding: 16 heads split across 8 cores (2 heads/core), both batch elements
# on every core. Each core computes its heads' attention and its partial
# output projection; the host sums the 8 bf16 partials (f32 accumulate) and
# adds proj_b. LoRA is folded into the qkv weights on the host.
#
# v2 engine layout (per core):
#  - ACT runs ONLY exp (softmax) plus two tiny Ln/Exp calls per half for
#    rstd = exp(-0.5*ln(var+eps)); both functions live in the same activation
#    table set so the table is loaded once (Sqrt would thrash the table).
#  - DVE owns all PSUM evacuations (qkv stage, qT/kT, proj output), LN stats
#    reductions, and the softmax normalization (reciprocal + multiply).
#  - Pool (gpsimd) applies the LN normalization (tensor_scalar) and builds
#    the [v|ones] tiles; it never touches PSUM.
#  - PE: qkv matmuls, q/k transposes (col-tile paired), scores (row-tile
#    paired across the 2 heads: head0 rows 0-63, head1 rows 64-127 of the
#    array run concurrently), attention@[v|ones], packed 128-contraction
#    projection.
#  - Phase A (qkv+LN+transpose) of batch b+1 is emitted interleaved into
#    phase B (attention) of batch b so PE/DVE fill the gaps while ACT grinds
#    exp, and PE never idles long enough to lose the HAM clock boost.
import numpy as np
import ml_dtypes

import concourse.bass as bass
import concourse.bacc as bacc
import concourse.mybir as mybir
from concourse.tile import TileContext
from concourse.masks import make_identity
from concourse.bass_utils import run_bass_kernel_spmd

BF16 = ml_dtypes.bfloat16
F32 = np.float32

B, N, DIM, H, R = 2, 2048, 1024, 16, 8
D = DIM // H              # 64
NCORES = 8
HPC = H // NCORES         # 2 heads per core
ALPHA = 8.0
LORA_SCALE = ALPHA / R
EPS = 1e-5
QSCALE = float(D) ** -0.5  # 0.125

NCH = DIM // 128          # 8 contraction chunks of 128
NTI = N // 128            # 16 row tiles of 128
QI = 512                  # query-block width (free dim of score matmuls)
NIB = N // QI             # 4 query blocks per batch
HT = NTI // 2             # 8 row tiles per LN-stats half

_prog_cache: dict = {}


def _build_program(use_mask: bool, affine_q: bool, affine_k: bool, repeat: int = 1):
    nc = bacc.Bacc("TRN2", target_bir_lowering=False)
    f32 = mybir.dt.float32
    bf16 = mybir.dt.bfloat16

    xT = nc.dram_tensor("xT", [128, B, NCH, N], bf16, kind="ExternalInput")
    wT = nc.dram_tensor("wT", [NCH, 128, 6 * D], bf16, kind="ExternalInput")
    projT = nc.dram_tensor("projT", [128, DIM], bf16, kind="ExternalInput")
    out_p = nc.dram_tensor("out_p", [128, B, NTI, DIM], bf16, kind="ExternalOutput")
    if affine_q or affine_k:
        lnaff = nc.dram_tensor("lnaff", [4, 128, D], f32, kind="ExternalInput")
    if use_mask:
        emaskT = nc.dram_tensor("emaskT", [N, N], bf16, kind="ExternalInput")

    with TileContext(nc) as tc:
        import contextlib
        with contextlib.ExitStack() as ctx:
            const = ctx.enter_context(tc.tile_pool(name="const", bufs=1))
            ident = const.tile([128, 128], bf16)
            make_identity(nc, ident)
            eps_t = const.tile([128, 1], f32)
            nc.vector.memset(eps_t, EPS)
            # Preload the one activation-table set that holds BOTH Ln and
            # Exp; the table-load fixpoint then sees every Ln/Exp covered on
            # all paths and inserts no further (thrashing) loads.
            from concourse.hw_specs import get_activation_tables
            try:
                _tnames = list(get_activation_tables(nc.m.arch).keys())
                _set_id = _tnames.index("natural_log_exp_and_others")
            except Exception:
                _set_id = 6
            nc.scalar.add_instruction(
                mybir.InstLoadActFuncSet(
                    name=nc.get_next_instruction_name(), ins=[], outs=[],
                    act_func_set_id=_set_id))

            persist = ctx.enter_context(tc.tile_pool(name="persist", bufs=1))
            w_sb = persist.tile([128, NCH, 6 * D], bf16)
            nc.sync.dma_start(out=w_sb, in_=wT.rearrange("ci cm w -> cm ci w"))
            proj_sb = persist.tile([128, DIM], bf16)
            nc.sync.dma_start(out=proj_sb, in_=projT[:, :])
            if affine_q or affine_k:
                aff_sb = persist.tile([128, 4, D], f32)
                nc.sync.dma_start(out=aff_sb, in_=lnaff.rearrange("r p d -> p r d"))
            # [v|ones] stationary tiles: head0 = [v, 1], head1 = [1, v] so the
            # attention output of head1 lands on partitions 64-127 (den on
            # 0-63) and both heads' normalized outputs pack into one
            # 128-partition lhsT for the projection matmul.
            vps = []
            for par in range(B):
                vp = persist.tile([128, NTI, HPC, 128], bf16, name=f"vp{par}")
                nc.vector.memset(vp[:, :, 0, D:], 1.0)
                nc.vector.memset(vp[:, :, 1, :D], 1.0)
                vps.append(vp)

            xpool = ctx.enter_context(tc.tile_pool(name="xpool", bufs=2))
            qkpool = ctx.enter_context(tc.tile_pool(name="qkpool", bufs=2))
            stg = ctx.enter_context(tc.tile_pool(name="stg", bufs=2))
            sqp = ctx.enter_context(tc.tile_pool(name="sqp", bufs=2))
            lnp = ctx.enter_context(tc.tile_pool(name="lnp", bufs=2))
            natp = ctx.enter_context(tc.tile_pool(name="natp", bufs=8))
            esp = ctx.enter_context(tc.tile_pool(name="esp", bufs=4))
            otp = ctx.enter_context(tc.tile_pool(name="otp", bufs=4))
            outp = ctx.enter_context(tc.tile_pool(name="outp", bufs=2))
            if use_mask:
                mskp = ctx.enter_context(tc.tile_pool(name="mskp", bufs=4))
            # PSUM budget (8 banks, per-tag): sT f32 2x2 + av/pp shared 2 +
            # pq 1 + pt 1 = 8
            psS = ctx.enter_context(tc.tile_pool(name="psS", bufs=2, space="PSUM"))
            psB = ctx.enter_context(tc.tile_pool(name="psB", bufs=2, space="PSUM"))
            psA = ctx.enter_context(tc.tile_pool(name="psA", bufs=2, space="PSUM"))

            if repeat > 1:
                ctx.enter_context(tc.For_i(
                    0, repeat, 1,
                    hint_engines=(mybir.EngineType.PE, mybir.EngineType.SP,
                                  mybir.EngineType.Activation,
                                  mybir.EngineType.DVE, mybir.EngineType.Pool)))

            # ---- hoisted input loads for both batches ----
            x_sbs = []
            for b in range(B):
                x_sb = xpool.tile([128, NCH, N], bf16, tag="x_sb")
                nc.sync.dma_start(out=x_sb[:, 0:NCH // 2, :],
                                  in_=xT[:, b, 0:NCH // 2, :])
                nc.sync.dma_start(out=x_sb[:, NCH // 2:, :],
                                  in_=xT[:, b, NCH // 2:, :])
                x_sbs.append(x_sb)

            # ---------------- phase A emission (as fine-grained steps) -----
            def make_A_steps(b, borrow_sT=False):
                """Returns (steps, handles); handles filled as steps run.
                borrow_sT: batch-0's phase A runs with no phase B interleaved
                under it, so its qkv accumulators borrow the (idle) sT psum
                tag to double-buffer instead of serializing on one bank."""
                hd = {}
                x_sb = x_sbs[b]

                def alloc():
                    qkT = qkpool.tile([128, 2, N], bf16, tag="qkT", name="qkT")
                    hd["qT"] = qkT[:, 0, :]
                    hd["kT"] = qkT[:, 1, :]
                    hd["qkT"] = qkT
                    hd["stage"] = stg.tile([128, NTI, 6 * D], bf16, tag="stage", name="stage")
                    hd["sqs"] = sqp.tile([128, NTI, 6 * D], bf16, tag="sqs", name="sqs")

                def qkv_ti(ti):
                    def f():
                        stage = hd["stage"]; sqs = hd["sqs"]
                        if borrow_sT:
                            pqt = psS.tile([128, 2, QI], f32, tag="sT", name="pqt")
                            pq = pqt[:, 0, :]
                        else:
                            pq = psA.tile([128, 512], f32, tag="pq", bufs=1)
                        for ci in range(NCH):
                            nc.tensor.matmul(
                                pq[:, 0:6 * D],
                                lhsT=x_sb[:, ci, ti * 128:(ti + 1) * 128],
                                rhs=w_sb[:, ci, :],
                                start=(ci == 0), stop=(ci == NCH - 1),
                            )
                        nc.vector.tensor_copy(out=stage[:, ti, :], in_=pq[:, 0:6 * D])
                        nc.vector.tensor_tensor(
                            out=sqs[:, ti, :], in0=stage[:, ti, :],
                            in1=stage[:, ti, :], op=mybir.AluOpType.mult)
                    return f

                def stats(half):
                    def f():
                        stage = hd["stage"]; sqs = hd["sqs"]
                        st6v = stage.rearrange("p t (i d) -> p t i d", d=D)
                        sq6v = sqs.rearrange("p t (i d) -> p t i d", d=D)
                        hsl = slice(half * HT, (half + 1) * HT)
                        mean = lnp.tile([128, HT, 6], bf16, tag="meanh")
                        with nc.allow_low_precision(reason="LN stats: bf16 out, fp32 internal accum"):
                            nc.vector.tensor_reduce(
                                out=mean, in_=st6v[:, hsl], axis=mybir.AxisListType.X,
                                op=mybir.AluOpType.add)
                        meanf = lnp.tile([128, HT, 6], f32, tag="mean")
                        nc.vector.tensor_scalar(
                            out=meanf, in0=mean, scalar1=1.0 / D, scalar2=None,
                            op0=mybir.AluOpType.mult)
                        varh = lnp.tile([128, HT, 6], bf16, tag="varh")
                        with nc.allow_low_precision(reason="LN stats: bf16 out, fp32 internal accum"):
                            nc.vector.tensor_reduce(
                                out=varh, in_=sq6v[:, hsl], axis=mybir.AxisListType.X,
                                op=mybir.AluOpType.add)
                        var = lnp.tile([128, HT, 6], f32, tag="var")
                        nc.vector.tensor_scalar(
                            out=var, in0=varh, scalar1=1.0 / D, scalar2=None,
                            op0=mybir.AluOpType.mult)
                        m2 = lnp.tile([128, HT, 6], f32, tag="m2")
                        nc.vector.tensor_tensor(
                            out=m2, in0=meanf, in1=meanf, op=mybir.AluOpType.mult)
                        nc.vector.tensor_tensor(
                            out=var, in0=var, in1=m2, op=mybir.AluOpType.subtract)
                        # rstd = exp(-0.5 * ln(var + eps)); Ln and Exp share
                        # one activation table set (unlike Sqrt).
                        lnv = lnp.tile([128, HT, 6], f32, tag="lnv")
                        nc.scalar.activation(
                            out=lnv, in_=var,
                            func=mybir.ActivationFunctionType.Ln,
                            bias=eps_t, scale=1.0)
                        rstd = lnp.tile([128, HT, 6], f32, tag="rstd")
                        nc.scalar.activation(
                            out=rstd, in_=lnv,
                            func=mybir.ActivationFunctionType.Exp,
                            scale=-0.5)
                        if not affine_q:
                            nc.vector.tensor_scalar(
                                out=rstd[:, :, 0:6:3], in0=rstd[:, :, 0:6:3],
                                scalar1=QSCALE, scalar2=None,
                                op0=mybir.AluOpType.mult)
                        hd[("mean", half)] = meanf
                        hd[("rstd", half)] = rstd
                    return f

                def natT_ti(half, tih):
                    def f():
                        stage = hd["stage"]
                        st6v = stage.rearrange("p t (i d) -> p t i d", d=D)
                        mean = hd[("mean", half)]; rstd = hd[("rstd", half)]
                        ti = half * HT + tih
                        nats = []
                        # order q_h0, q_h1, k_h0, k_h1 so the two transposes
                        # of each pair hit col-groups (0,*) and (64,*)
                        for inst, qk, hh in ((0, 0, 0), (3, 0, 1),
                                             (1, 1, 0), (4, 1, 1)):
                            affine = affine_q if qk == 0 else affine_k
                            nat = natp.tile([128, D], bf16, tag="nat")
                            if affine:
                                natf = natp.tile([128, D], f32, tag="natf")
                                nc.gpsimd.tensor_scalar(
                                    out=natf, in0=st6v[:, ti, inst, :],
                                    scalar1=mean[:, tih, inst:inst + 1],
                                    scalar2=rstd[:, tih, inst:inst + 1],
                                    op0=mybir.AluOpType.subtract,
                                    op1=mybir.AluOpType.mult)
                                r = 0 if qk == 0 else 2
                                natf2 = natp.tile([128, D], f32, tag="natf2")
                                nc.gpsimd.tensor_tensor(
                                    out=natf2, in0=natf, in1=aff_sb[:, r, :],
                                    op=mybir.AluOpType.mult)
                                nc.gpsimd.tensor_tensor(
                                    out=nat, in0=natf2, in1=aff_sb[:, r + 1, :],
                                    op=mybir.AluOpType.add)
                            else:
                                nc.gpsimd.tensor_scalar(
                                    out=nat, in0=st6v[:, ti, inst, :],
                                    scalar1=mean[:, tih, inst:inst + 1],
                                    scalar2=rstd[:, tih, inst:inst + 1],
                                    op0=mybir.AluOpType.subtract,
                                    op1=mybir.AluOpType.mult)
                            nats.append((nat, qk, hh))
                        pt = psA.tile([128, 2, 512], bf16, tag="pt", bufs=1)
                        for nat, qk, hh in nats:
                            nc.tensor.transpose(
                                pt[hh * D:(hh + 1) * D, qk, 0:128], nat, ident)
                        nc.vector.tensor_copy(
                            out=hd["qkT"][:, :, ti * 128:(ti + 1) * 128],
                            in_=pt[:, :, 0:128])
                        nc.gpsimd.tensor_copy(
                            out=vps[b][:, ti, 0, 0:D], in_=st6v[:, ti, 2, :])
                        nc.gpsimd.tensor_copy(
                            out=vps[b][:, ti, 1, D:], in_=st6v[:, ti, 5, :])
                    return f

                steps = [alloc]
                for half in range(2):
                    for tih in range(HT):
                        steps.append(qkv_ti(half * HT + tih))
                    steps.append(stats(half))
                # natT after BOTH halves' qkv so the PE stream never gaps
                # on the (DVE-only) stats steps long enough to lose HAM
                for half in range(2):
                    for tih in range(HT):
                        steps.append(natT_ti(half, tih))
                return steps, hd

            def run_all(steps):
                for s in steps:
                    s()

            # ---------------- phase B emission ------------------------------
            def emit_B(b, hd, filler, rounds=range(NIB)):
                def pull(n=1):
                    for _ in range(n):
                        s = next(filler, None)
                        if s is not None:
                            s()
                qT = hd["qT"]; kT = hd["kT"]; vp = vps[b]
                def emit_scores(ib, jp):
                    i0 = ib * QI
                    sTs = []
                    for hh in range(HPC):
                        sTs.append(psS.tile([128, 2, QI], f32, tag="sT", name="sT"))
                    for cj in range(2):
                        j = jp * 2 + cj
                        for hh in range(HPC):
                            hs = slice(hh * D, (hh + 1) * D)
                            nc.tensor.matmul(
                                sTs[hh][:, cj, :],
                                lhsT=kT[hs, j * 128:(j + 1) * 128],
                                rhs=qT[hs, i0:i0 + QI],
                                start=True, stop=True,
                            )
                    return sTs

                for ib in rounds:
                    i0 = ib * QI
                    avs = []
                    for hh in range(HPC):
                        av = psB.tile([128, QI], f32, tag="av", name="av")
                        avs.append(av)
                    sTs = emit_scores(ib, 0)
                    for jp in range(8):
                        ess = []
                        for hh in range(HPC):
                            ess.append(esp.tile([128, 2, QI], bf16, tag="es", name="es"))
                        for hh in range(HPC):
                            nc.scalar.activation(
                                out=ess[hh], in_=sTs[hh],
                                func=mybir.ActivationFunctionType.Exp)
                        if use_mask:
                            for hh in range(HPC):
                                msk = mskp.tile([128, 2, QI], bf16, tag="msk")
                                for cj in range(2):
                                    j = jp * 2 + cj
                                    nc.sync.dma_start(
                                        out=msk[:, cj, :],
                                        in_=emaskT[j * 128:(j + 1) * 128,
                                                   i0:i0 + QI])
                                nc.vector.tensor_tensor(
                                    out=ess[hh], in0=ess[hh], in1=msk,
                                    op=mybir.AluOpType.mult)
                        # next jp's scores go on the PE queue AHEAD of the
                        # exp-gated av matmuls so ACT never starves
                        if jp + 1 < 8:
                            sTs = emit_scores(ib, jp + 1)
                        pull(1)
                        for hh in range(HPC):
                            for cj in range(2):
                                j = jp * 2 + cj
                                nc.tensor.matmul(
                                    avs[hh],
                                    lhsT=vp[:, j, hh, :],
                                    rhs=ess[hh][:, cj, :],
                                    start=(j == 0), stop=(j == NTI - 1),
                                )
                    # normalization: av0 = [out|den], av1 = [den|out]
                    zr = otp.tile([128, QI], f32, tag="zr")
                    nc.vector.reciprocal(out=zr[0:D, :], in_=avs[0][D:, :])
                    nc.vector.reciprocal(out=zr[D:, :], in_=avs[1][0:D, :])
                    oT2 = otp.tile([128, QI], bf16, tag="oT2")
                    nc.vector.tensor_tensor(
                        out=oT2[0:D, :], in0=avs[0][0:D, :], in1=zr[0:D, :],
                        op=mybir.AluOpType.mult)
                    nc.vector.tensor_tensor(
                        out=oT2[D:, :], in0=avs[1][D:, :], in1=zr[D:, :],
                        op=mybir.AluOpType.mult)
                    pull(1)
                    osb = outp.tile([128, QI // 128, DIM], bf16, tag="osb")
                    for sub in range(QI // 128):
                        for nh in range(2):
                            pp = psB.tile([128, 512], f32, tag="av", name="pp")
                            nc.tensor.matmul(
                                pp,
                                lhsT=oT2[:, sub * 128:(sub + 1) * 128],
                                rhs=proj_sb[:, nh * 512:(nh + 1) * 512],
                                start=True, stop=True,
                            )
                            nc.vector.tensor_copy(
                                out=osb[:, sub, nh * 512:(nh + 1) * 512], in_=pp)
                        pull(1)
                    ti0 = ib * (QI // 128)
                    nc.sync.dma_start(
                        out=out_p[:, b, ti0:ti0 + QI // 128, :], in_=osb)

            # ---------------- schedule --------------------------------------
            stepsA0, hd0 = make_A_steps(0, borrow_sT=True)
            run_all(stepsA0)
            stepsA1, hd1 = make_A_steps(1)
            fillA1 = iter(stepsA1)
            emit_B(0, hd0, fillA1, rounds=range(NIB - 1))
            run_all(fillA1)  # drain A1 steps not yet pulled
            # stripe the batch boundary: B1's first round interleaves with
            # B0's last so ACT keeps a ready exp stream across the switch
            emit_B(1, hd1, iter(()), rounds=range(1))
            emit_B(0, hd0, iter(()), rounds=range(NIB - 1, NIB))
            emit_B(1, hd1, iter(()), rounds=range(1, NIB))
    nc.compile()
    return nc


def _prep_inputs(inputs):
    x = np.ascontiguousarray(inputs["x"], dtype=F32)
    qkv_w = np.asarray(inputs["qkv_w"], dtype=F32)
    proj_w = np.asarray(inputs["proj_w"], dtype=F32)
    W_eff = qkv_w.copy()
    for i, (a, bm) in enumerate([("lora_Aq", "lora_Bq"), ("lora_Ak", "lora_Bk"),
                                 ("lora_Av", "lora_Bv")]):
        A = np.asarray(inputs[a], dtype=F32)
        Bm = np.asarray(inputs[bm], dtype=F32)
        W_eff[i * DIM:(i + 1) * DIM] += LORA_SCALE * (A @ Bm).T

    xT_all = np.ascontiguousarray(
        x.transpose(2, 0, 1).reshape(NCH, 128, B, N)
        .transpose(1, 2, 0, 3).astype(BF16))

    qn_w = np.asarray(inputs["qn_w"], F32); qn_b = np.asarray(inputs["qn_b"], F32)
    kn_w = np.asarray(inputs["kn_w"], F32); kn_b = np.asarray(inputs["kn_b"], F32)
    affine_q = not (np.all(qn_w == 1.0) and np.all(qn_b == 0.0))
    affine_k = not (np.all(kn_w == 1.0) and np.all(kn_b == 0.0))
    mask = np.asarray(inputs["attn_mask"], F32)
    use_mask = bool(np.any(mask))

    common = {"xT": xT_all}
    if affine_q or affine_k:
        aff = np.stack([
            np.broadcast_to(qn_w * QSCALE, (128, D)),
            np.broadcast_to(qn_b * QSCALE, (128, D)),
            np.broadcast_to(kn_w, (128, D)),
            np.broadcast_to(kn_b, (128, D)),
        ]).astype(F32)
        common["lnaff"] = np.ascontiguousarray(aff)
    if use_mask:
        common["emaskT"] = np.ascontiguousarray(
            np.exp(mask[0, 0].T).astype(BF16))

    in_maps = []
    for c in range(NCORES):
        h0 = c * HPC
        blocks = []
        for hh in range(HPC):
            h = h0 + hh
            for part in range(3):  # q, k, v
                blocks.append(W_eff[part * DIM + h * D: part * DIM + (h + 1) * D])
        Wlocal = np.concatenate(blocks, axis=0)          # [384, 1024]
        wT_c = np.ascontiguousarray(
            Wlocal.T.reshape(NCH, 128, 6 * D).astype(BF16))
        projT_c = np.ascontiguousarray(np.concatenate(
            [proj_w[:, (h0 + hh) * D:(h0 + hh + 1) * D].T for hh in range(HPC)],
            axis=0).astype(BF16))                        # [128, 1024]
        m = dict(common)
        m["wT"] = wT_c
        m["projT"] = projT_c
        in_maps.append(m)
    return in_maps, (use_mask, affine_q, affine_k)


def _run(inputs, trace=False):
    in_maps, key = _prep_inputs(inputs)
    if key not in _prog_cache:
        _prog_cache[key] = _build_program(*key)
    nc = _prog_cache[key]
    res = run_bass_kernel_spmd(nc, in_maps, core_ids=list(range(NCORES)),
                               trace=trace)
    acc = np.zeros((128, B, NTI, DIM), dtype=F32)
    for r in res.results:
        acc += r["out_p"].astype(F32)
    out = np.ascontiguousarray(acc.transpose(1, 2, 0, 3).reshape(B, N, DIM))
    out += np.asarray(inputs["proj_b"], F32)
    return out, res


def kernel(**inputs) -> np.ndarray:
    out, _ = _run(inputs)
    return out
